# revision 1
# baseline (speedup 1.0000x reference)
"""Trainium2 Bass kernel for CustomConvWithExtra.

out = conv3x3(x, w_main) + b_main + extra, where extra collapses to a 3x3
border-class table T[b,c,clsh,clsw] (conv of a spatially-constant image).

Design (v6):
 - Data parallel: 1 batch image per NeuronCore (B=8 = 8 cores).
 - Per output ROW-PAIR: 3 accumulating matmuls (one per kw tap column) into a
   single PSUM bank [128,512]; kw is applied by sliding the rhs window along
   the patch free dim (patch rows are 514 wide). float32r -> full-rate PE.
 - Patch compression: rows for (pair,kh) with equal d=pair+kh are identical,
   so the patch holds only 12 distinct rows (d in 0..3 x ci) + 3 static rows
   (w==0 indicator, w==W-1 indicator, ones) that fuse bias+border terms into
   the kw=1 matmul.  lhsT row (d,ci) of pair-block p carries wm[:,ci,d-p,kw].
 - Host pre-arranges shifted row-planes xp_rep[d,ci,r,:] = xp[ci,r+d,:] so the
   whole per-chunk patch fill is ONE 3-dim DMA [12, C*514] (single stride
   across the 12 (d,ci) planes; descriptors spread over SDMA engines).
 - Output: ob [128, C*512] (partition = pair*64+ch), 2 SWDGE (gpsimd) DMAs per
   chunk (64 partitions each) so output traffic drains from a separate DMA
   ring than the patch fills (no head-of-line blocking).
"""

from contextlib import ExitStack

import numpy as np

import concourse.bass as bass
import concourse.tile as tile
from concourse import bacc, mybir
from concourse.bass_utils import run_bass_kernel_spmd

# Problem shapes (hardcoded per contract)
B, CIN, H, W = 8, 3, 512, 512
COUT, E, KS = 64, 3, 3
NCORES = 8
KP = 15            # patch partitions: 12 = (d,ci) + indL + indR + ones
C = 16             # row-pairs per chunk
F32R = mybir.dt.float32r
F32 = mybir.dt.float32

_cache: dict = {}


def _build(h: int = H, w: int = W):
    xrow = w + 2
    pairs = h // 2
    c = min(C, pairs)
    nchunk = pairs // c
    assert pairs % c == 0

    nc = bacc.Bacc("TRN2", target_bir_lowering=False, debug=False)
    # xp_rep[q, r, :] with q = d*CIN+ci: xp[ci, r+d, :] (zero where OOB)
    xrep = nc.dram_tensor("xrep", [4 * CIN, h, xrow], F32R, kind="ExternalInput").ap()
    wts = nc.dram_tensor("wts", [9, KP, 128], F32R, kind="ExternalInput").ap()
    stat = nc.dram_tensor("stat", [3, c * xrow], F32R, kind="ExternalInput").ap()
    out = nc.dram_tensor("out", [COUT, h, w], F32, kind="ExternalOutput").ap()

    PBUFS = 3
    with tile.TileContext(nc) as tc, ExitStack() as ctx:
        wpool = ctx.enter_context(tc.tile_pool(name="wpool", bufs=1))
        ppool = ctx.enter_context(tc.tile_pool(name="ppool", bufs=PBUFS))
        opool = ctx.enter_context(tc.tile_pool(name="opool", bufs=3))
        pspool = ctx.enter_context(tc.tile_pool(name="pspool", bufs=8, space="PSUM"))

        # Stationary weights: wtile[k, u*128+m] = wts[u, k, m], u = vrow*3+kw
        wtile = wpool.tile([KP, 9 * 128], F32R)
        nc.sync.dma_start(
            wtile[:, :],
            bass.AP(wts.tensor, 0, [[128, KP], [KP * 128, 9], [1, 128]]),
        )

        # Patch buffers; static rows 12:15 loaded once per physical buffer.
        patch_tiles = []
        for s in range(PBUFS):
            pt = ppool.tile([KP, c * xrow], F32R, name=f"patch{s}", tag="patch")
            nc.sync.dma_start(pt[12:15, :], stat[:, :])
            patch_tiles.append(pt)

        for ch in range(nchunk):
            pt = patch_tiles[ch % PBUFS]
            h0 = ch * c * 2  # first output row of chunk
            # ONE DMA: rows q=(d,ci) <- xp_rep[q, h0+2j, :]
            src = bass.AP(
                xrep.tensor,
                h0 * xrow,
                [[h * xrow, 4 * CIN], [2 * xrow, c], [1, xrow]],
            )
            nc.sync.dma_start(pt[0:12, :], src)

            ob = opool.tile([128, c * w], F32, name="ob", tag="ob")
            for j in range(c):
                pairidx = ch * c + j
                vrow = 0 if pairidx == 0 else (2 if pairidx == pairs - 1 else 1)
                ps = pspool.tile([128, w], F32, name="ps", tag="ps")
                for kw in range(3):
                    u = vrow * 3 + kw
                    nc.tensor.matmul(
                        ps[:, :],
                        wtile[:, u * 128 : (u + 1) * 128],
                        pt[:, j * xrow + kw : j * xrow + kw + w],
                        start=(kw == 0),
                        stop=(kw == 2),
                    )
                if j % 2 == 0:
                    nc.vector.tensor_copy(ob[:, j * w : (j + 1) * w], ps[:, :])
                else:
                    nc.scalar.copy(ob[:, j * w : (j + 1) * w], ps[:, :])

            for pair in range(2):
                dst = bass.AP(
                    out.tensor,
                    (h0 + pair) * w,
                    [[h * w, COUT], [2 * w, c], [1, w]],
                )
                nc.gpsimd.dma_start(dst, ob[pair * 64 : (pair + 1) * 64, :])

    nc.compile()
    return nc


def _host_prep(x, v, wm, bm, we, be, h=H, w=W, c=C):
    """Per-core inputs: shifted row-planes, fused weight variants, statics."""
    Bb = x.shape[0]
    vr = v.reshape(Bb, COUT, E).astype(np.float64)

    sets = {0: [1, 2], 1: [0, 1, 2], 2: [0, 1]}
    Mcl = np.zeros((COUT, E, 3, 3), np.float64)
    we64 = we.astype(np.float64)
    for ch_ in range(3):
        for cw in range(3):
            Mcl[:, :, ch_, cw] = we64[:, :, sets[ch_], :][:, :, :, sets[cw]].sum((2, 3))
    T = (
        np.einsum("bce,cehw->bchw", vr, Mcl)
        + bm.astype(np.float64)[None, :, None, None]
        + be.astype(np.float64)[None, :, None, None]
    )

    xrow = w + 2
    xp = np.pad(x, ((0, 0), (0, 0), (1, 1), (1, 1))).astype(np.float32)
    # xrep[b, d*CIN+ci, r, :] = xp[b, ci, r+d, :]  (r in [0,h), zero where OOB)
    xrep = np.zeros((Bb, 4 * CIN, h, xrow), np.float32)
    for d in range(4):
        nr = min(h, h + 2 - d)
        xrep[:, d * CIN : (d + 1) * CIN, :nr, :] = xp[:, :, d : d + nr, :]

    # vrow: 0 = pair (rows 0,1) classes (top,mid); 1 = interior; 2 = (mid,bot)
    pair_cls = {0: (0, 1), 1: (1, 1), 2: (1, 2)}
    wts = np.zeros((Bb, 9, KP, 128), np.float32)
    for b in range(Bb):
        for vrow in range(3):
            for kw in range(KS):
                u = vrow * 3 + kw
                for pair in range(2):
                    cols = slice(pair * 64, pair * 64 + 64)
                    for d in range(4):
                        kh = d - pair
                        if 0 <= kh < KS:
                            for ci in range(CIN):
                                wts[b, u, d * CIN + ci, cols] = wm[:, ci, kh, kw]
                    if kw == 1:  # statics only fire in the center-kw matmul
                        cls = pair_cls[vrow][pair]
                        wts[b, u, 12, cols] = T[b, :, cls, 0] - T[b, :, cls, 1]
                        wts[b, u, 13, cols] = T[b, :, cls, 2] - T[b, :, cls, 1]
                        wts[b, u, 14, cols] = T[b, :, cls, 1]

    stat = np.zeros((3, c * xrow), np.float32)
    stat[0, 1::xrow] = 1.0          # rhs col 0 under kw=1 window
    stat[1, w::xrow] = 1.0          # rhs col w-1 under kw=1 window
    stat[2, :] = 1.0                # ones row (base bias)
    return xrep, wts, stat


def kernel(**inputs) -> np.ndarray:
    x = np.ascontiguousarray(np.asarray(inputs["x"], np.float32))
    v = np.asarray(inputs["extra_inputs"], np.float32)
    wm = np.asarray(inputs["w_main"], np.float32)
    bm = np.asarray(inputs["b_main"], np.float32)
    we = np.asarray(inputs["w_extra"], np.float32)
    be = np.asarray(inputs["b_extra"], np.float32)

    xrep, wts, stat = _host_prep(x, v, wm, bm, we, be)

    if "nc" not in _cache:
        _cache["nc"] = _build()
    nc = _cache["nc"]

    in_maps = [{"xrep": xrep[b], "wts": wts[b], "stat": stat} for b in range(B)]
    res = run_bass_kernel_spmd(nc, in_maps, list(range(NCORES)))
    return np.stack([res.results[b]["out"] for b in range(B)]).astype(np.float32)



# revision 2
# speedup vs baseline: 1.4352x; 1.4352x over previous
"""Trainium2 Bass kernel for CustomConvWithExtra.

out = conv3x3(x, w_main) + b_main + extra, where extra collapses to a 3x3
border-class table T[b,c,clsh,clsw] (conv of a spatially-constant image).

Design (v7):
 - Data parallel: 1 batch image per NeuronCore (B=8 = 8 cores).
 - fp16 end-to-end on the wire: x is sent as fp16, output is written as fp16
   and upcast to f32 on the host (tolerance is 2e-2; fp16 round-off ~4e-4).
   This halves the dominant HBM write traffic (67MB -> 33.5MB per core).
 - Per output ROW-PAIR: ONE matmul. All 3 kw tap columns are packed into the
   contraction dim: patch rows (kw, d, ci) with d = pair+kh in 0..3, 36 rows
   + 3 static rows (col-0 indicator, col-(W-1) indicator, ones) that fuse
   bias+border terms = 39-row contraction, 128 output partitions
   (pair*64+ch), free dim W=512.  fp16 PE runs 1 row/cycle at 2.4GHz.
 - The kw=1,2 row replicas are read from the SAME HBM planes at +1/+2
   element offsets: plane p holds rows (ch,j) flattened to c*514-blocks, and
   the matmul window for pair j only reads cols [j*514, j*514+512), never the
   last 2 cols of a block, so a flat shifted read gives exactly the
   column-shifted rows (block-crossing garbage lands in unread cols).
 - Output: ob [128, c*512] fp16; HBM layout out[p, ch*8192 + j*512 + x] so
   each chunk's store is ONE DMA with 128 contiguous 16KB descriptors
   (vs 2KB strided lines before -> descriptor overhead dominated).
   Host reassembles [64, 512, 512] and upcasts.
"""

from contextlib import ExitStack

import numpy as np

import concourse.bass as bass
import concourse.tile as tile
from concourse import bacc, mybir
from concourse.bass_utils import run_bass_kernel_spmd

# Problem shapes (hardcoded per contract)
B, CIN, H, W = 8, 3, 512, 512
COUT, E, KS = 64, 3, 3
NCORES = 8
XROW = W + 2       # 514
KP = 39            # patch partitions: 36 = (kw,d,ci) + indL + indR + ones
C = 16             # row-pairs per chunk
PAIRS = H // 2
NCHUNK = PAIRS // C
PLANE = PAIRS * XROW          # 131584 elems per (d,ci) plane
PLANE_PAD = PLANE + 4         # slack so the +2 shifted read of the last plane stays in-bounds
CBLK = C * XROW               # 8224 patch cols per chunk
OBLK = C * W                  # 8192 output cols per chunk
F16 = mybir.dt.float16
F32 = mybir.dt.float32

_cache: dict = {}


def _build():
    nc = bacc.Bacc("TRN2", target_bir_lowering=False, debug=False)
    # xrep[p, ch*CBLK + j*XROW + col] = xp[ci, 2*(ch*C+j)+d, col], p = d*CIN+ci
    xrep = nc.dram_tensor("xrep", [4 * CIN, PLANE_PAD], F16, kind="ExternalInput").ap()
    wts = nc.dram_tensor("wts", [KP, 3 * 128], F16, kind="ExternalInput").ap()
    stat = nc.dram_tensor("stat", [3, CBLK], F16, kind="ExternalInput").ap()
    out = nc.dram_tensor("out", [128, NCHUNK * OBLK], F16, kind="ExternalOutput").ap()

    PBUFS = 3
    with tile.TileContext(nc) as tc, ExitStack() as ctx:
        wpool = ctx.enter_context(tc.tile_pool(name="wpool", bufs=1))
        ppool = ctx.enter_context(tc.tile_pool(name="ppool", bufs=PBUFS))
        opool = ctx.enter_context(tc.tile_pool(name="opool", bufs=3))
        pspool = ctx.enter_context(tc.tile_pool(name="pspool", bufs=8, space="PSUM"))

        # Stationary weights: wtile[k, u*128 + pair*64 + co], u = row-class
        wtile = wpool.tile([KP, 3 * 128], F16)
        nc.sync.dma_start(wtile[:, :], wts[:, :])

        # Patch buffers; static rows 36:39 loaded once per physical buffer.
        patch_tiles = []
        for s in range(PBUFS):
            pt = ppool.tile([KP, CBLK], F16, name=f"patch{s}", tag="patch")
            nc.sync.dma_start(pt[36:39, :], stat[:, :])
            patch_tiles.append(pt)

        for ch in range(NCHUNK):
            pt = patch_tiles[ch % PBUFS]
            # 3 DMAs: rows (kw,d,ci) <- plane (d,ci) shifted by kw elems
            for kw in range(3):
                src = bass.AP(
                    xrep.tensor,
                    ch * CBLK + kw,
                    [[PLANE_PAD, 4 * CIN], [1, CBLK]],
                )
                nc.sync.dma_start(pt[12 * kw : 12 * kw + 12, :], src)

            ob = opool.tile([128, OBLK], F16, name="ob", tag="ob")
            for j in range(C):
                pairidx = ch * C + j
                u = 0 if pairidx == 0 else (2 if pairidx == PAIRS - 1 else 1)
                ps = pspool.tile([128, W], F32, name="ps", tag="ps")
                nc.tensor.matmul(
                    ps[:, :],
                    wtile[:, u * 128 : (u + 1) * 128],
                    pt[:, j * XROW : j * XROW + W],
                    start=True,
                    stop=True,
                )
                if j % 2 == 0:
                    nc.vector.tensor_copy(ob[:, j * W : (j + 1) * W], ps[:, :])
                else:
                    nc.scalar.copy(ob[:, j * W : (j + 1) * W], ps[:, :])

            dst = bass.AP(
                out.tensor,
                ch * OBLK,
                [[NCHUNK * OBLK, 128], [1, OBLK]],
            )
            nc.sync.dma_start(dst, ob[:, :])

    nc.compile()
    return nc


def _host_prep(x, v, wm, bm, we, be):
    """Per-core inputs: shifted row-planes (fp16), fused weights, statics."""
    Bb = x.shape[0]
    vr = v.reshape(Bb, COUT, E).astype(np.float64)

    # Border-class table for the 'extra' convs of a constant image.
    sets = {0: [1, 2], 1: [0, 1, 2], 2: [0, 1]}
    Mcl = np.zeros((COUT, E, 3, 3), np.float64)
    we64 = we.astype(np.float64)
    for ch_ in range(3):
        for cw in range(3):
            Mcl[:, :, ch_, cw] = we64[:, :, sets[ch_], :][:, :, :, sets[cw]].sum((2, 3))
    T = (
        np.einsum("bce,cehw->bchw", vr, Mcl)
        + bm.astype(np.float64)[None, :, None, None]
        + be.astype(np.float64)[None, :, None, None]
    )

    # xrep[b, d*CIN+ci, (ch*C+j)*XROW + col] = xp[b, ci, 2*(ch*C+j)+d, col]
    xp = np.zeros((Bb, CIN, H + 2, XROW), np.float16)
    xp[:, :, 1 : H + 1, 1 : W + 1] = x.astype(np.float16)
    xrep = np.zeros((Bb, 4 * CIN, PLANE_PAD), np.float16)
    view = xrep[:, :, :PLANE].reshape(Bb, 4, CIN, PAIRS, XROW)
    for d in range(4):
        view[:, d] = xp[:, :, d : d + H : 2, :]

    # Fused weights: row q = kw*12 + d*CIN + ci, col u*128 + pair*64 + co
    pair_cls = {0: (0, 1), 1: (1, 1), 2: (1, 2)}
    wts = np.zeros((Bb, KP, 3, 128), np.float64)
    for u in range(3):
        for kw in range(KS):
            for pair in range(2):
                cols = slice(pair * 64, pair * 64 + 64)
                for d in range(4):
                    kh = d - pair
                    if 0 <= kh < KS:
                        for ci in range(CIN):
                            wts[:, kw * 12 + d * CIN + ci, u, cols] = wm[:, ci, kh, kw]
                cls = pair_cls[u][pair]
                wts[:, 36, u, cols] = T[:, :, cls, 0] - T[:, :, cls, 1]
                wts[:, 37, u, cols] = T[:, :, cls, 2] - T[:, :, cls, 1]
                wts[:, 38, u, cols] = T[:, :, cls, 1]
    wts = wts.reshape(Bb, KP, 3 * 128).astype(np.float16)

    stat = np.zeros((3, CBLK), np.float16)
    stat[0, 0::XROW] = 1.0          # rhs col 0 of each window
    stat[1, W - 1 :: XROW] = 1.0    # rhs col W-1 of each window
    stat[2, :] = 1.0                # ones row (bias + interior border term)
    return xrep, wts, stat


def kernel(**inputs) -> np.ndarray:
    x = np.ascontiguousarray(np.asarray(inputs["x"], np.float32))
    v = np.asarray(inputs["extra_inputs"], np.float32)
    wm = np.asarray(inputs["w_main"], np.float32)
    bm = np.asarray(inputs["b_main"], np.float32)
    we = np.asarray(inputs["w_extra"], np.float32)
    be = np.asarray(inputs["b_extra"], np.float32)

    xrep, wts, stat = _host_prep(x, v, wm, bm, we, be)

    if "nc" not in _cache:
        _cache["nc"] = _build()
    nc = _cache["nc"]

    in_maps = [{"xrep": xrep[b], "wts": wts[b], "stat": stat} for b in range(B)]
    res = run_bass_kernel_spmd(nc, in_maps, list(range(NCORES)))
    outs = []
    for b in range(B):
        ob = res.results[b]["out"]  # [128, NCHUNK*OBLK] fp16
        ob = ob.reshape(2, 64, NCHUNK, C, W).transpose(1, 2, 3, 0, 4)
        outs.append(ob.reshape(COUT, H, W).astype(np.float32))
    return np.stack(outs)


# revision 8
# speedup vs baseline: 2.4771x; 1.7260x over previous
"""Trainium2 Bass kernel for CustomConvWithExtra.

out = conv3x3(x, w_main) + b_main + extra, where extra collapses to a 3x3
border-class table T[b,c,clsh,clsw] (conv of a spatially-constant image).

Design (v7):
 - Data parallel: 1 batch image per NeuronCore (B=8 = 8 cores).
 - fp16 end-to-end on the wire: x is sent as fp16, output is written as fp16
   and upcast to f32 on the host (tolerance is 2e-2; fp16 round-off ~4e-4).
   This halves the dominant HBM write traffic (67MB -> 33.5MB per core).
 - Per output ROW-PAIR: ONE matmul. All 3 kw tap columns are packed into the
   contraction dim: patch rows (kw, d, ci) with d = pair+kh in 0..3, 36 rows
   + 3 static rows (col-0 indicator, col-(W-1) indicator, ones) that fuse
   bias+border terms = 39-row contraction, 128 output partitions
   (pair*64+ch), free dim W=512.  fp16 PE runs 1 row/cycle at 2.4GHz.
 - The kw=1,2 row replicas are read from the SAME HBM planes at +1/+2
   element offsets: plane p holds rows (ch,j) flattened to c*514-blocks, and
   the matmul window for pair j only reads cols [j*514, j*514+512), never the
   last 2 cols of a block, so a flat shifted read gives exactly the
   column-shifted rows (block-crossing garbage lands in unread cols).
 - Output: ob [128, c*512] fp16; HBM layout out[p, ch*8192 + j*512 + x] so
   each chunk's store is ONE DMA with 128 contiguous 16KB descriptors
   (vs 2KB strided lines before -> descriptor overhead dominated).
   Host reassembles [64, 512, 512] and upcasts.
"""

from contextlib import ExitStack

import numpy as np

import concourse.bass as bass
import concourse.tile as tile
from concourse import bacc, mybir
from concourse.bass_utils import run_bass_kernel_spmd

# Problem shapes (hardcoded per contract)
B, CIN, H, W = 8, 3, 512, 512
COUT, E, KS = 64, 3, 3
NCORES = 8
XROW = W + 2       # 514
KP = 39            # patch partitions: 36 = (kw,d,ci) + indL + indR + ones
C = 16             # row-pairs per chunk
PAIRS = H // 2
NCHUNK = PAIRS // C
PLANE = PAIRS * XROW          # 131584 elems per (d,ci) plane
PLANE_PAD = PLANE + 4         # slack so the +2 shifted read of the last plane stays in-bounds
CBLK = C * XROW               # 8224 patch cols per chunk
OBLK = C * W                  # 8192 output cols per chunk
F16 = mybir.dt.float16
F32 = mybir.dt.float32

_cache: dict = {}


def _build():
    nc = bacc.Bacc("TRN2", target_bir_lowering=False, debug=False)
    # xrep[p, ch*CBLK + j*XROW + col] = xp[ci, 2*(ch*C+j)+d, col], p = d*CIN+ci
    xrep = nc.dram_tensor("xrep", [4 * CIN, PLANE_PAD], F16, kind="ExternalInput").ap()
    wts = nc.dram_tensor("wts", [KP, 3 * 128], F16, kind="ExternalInput").ap()
    stat = nc.dram_tensor("stat", [3, CBLK], F16, kind="ExternalInput").ap()
    out = nc.dram_tensor("out", [128, NCHUNK * OBLK], F16, kind="ExternalOutput").ap()

    PBUFS = 4
    with tile.TileContext(nc) as tc, ExitStack() as ctx:
        wpool = ctx.enter_context(tc.tile_pool(name="wpool", bufs=1))
        ppool = ctx.enter_context(tc.tile_pool(name="ppool", bufs=PBUFS))
        opool = ctx.enter_context(tc.tile_pool(name="opool", bufs=4))
        pspool = ctx.enter_context(tc.tile_pool(name="pspool", bufs=8, space="PSUM"))

        # Stationary weights: wtile[k, u*128 + pair*64 + co], u = row-class
        wtile = wpool.tile([KP, 3 * 128], F16)
        nc.sync.dma_start(wtile[:, :], wts[:, :])

        # Patch buffers; static rows 36:39 loaded once per physical buffer.
        patch_tiles = []
        for s in range(PBUFS):
            pt = ppool.tile([KP, CBLK], F16, name=f"patch{s}", tag="patch")
            nc.sync.dma_start(pt[36:39, :], stat[:, :])
            patch_tiles.append(pt)

        for ch in range(NCHUNK):
            pt = patch_tiles[ch % PBUFS]
            # 3 DMAs: rows (kw,d,ci) <- plane (d,ci) shifted by kw elems
            for kw in range(3):
                src = bass.AP(
                    xrep.tensor,
                    ch * CBLK + kw,
                    [[PLANE_PAD, 4 * CIN], [1, CBLK]],
                )
                nc.sync.dma_start(pt[12 * kw : 12 * kw + 12, :], src)

            ob = opool.tile([128, OBLK], F16, name="ob", tag="ob")
            for j in range(C):
                pairidx = ch * C + j
                u = 0 if pairidx == 0 else (2 if pairidx == PAIRS - 1 else 1)
                ps = pspool.tile([128, W], F32, name="ps", tag="ps")
                nc.tensor.matmul(
                    ps[:, :],
                    wtile[:, u * 128 : (u + 1) * 128],
                    pt[:, j * XROW : j * XROW + W],
                    start=True,
                    stop=True,
                )
                if j % 2 == 0:
                    nc.vector.tensor_copy(ob[:, j * W : (j + 1) * W], ps[:, :])
                else:
                    nc.scalar.copy(ob[:, j * W : (j + 1) * W], ps[:, :])

            dst = bass.AP(
                out.tensor,
                ch * OBLK,
                [[NCHUNK * OBLK, 128], [1, OBLK]],
            )
            # Issued from the otherwise-idle gpsimd sequencer: its wait-for-ob
            # blocks nobody (sync only prefetches input, vector/scalar only
            # drain PSUM).
            nc.gpsimd.dma_start(dst, ob[:, :])

    nc.compile()
    return nc


def _host_prep(x, v, wm, bm, we, be):
    """Per-core inputs: shifted row-planes (fp16), fused weights, statics."""
    Bb = x.shape[0]
    vr = v.reshape(Bb, COUT, E).astype(np.float64)

    # Border-class table for the 'extra' convs of a constant image.
    sets = {0: [1, 2], 1: [0, 1, 2], 2: [0, 1]}
    Mcl = np.zeros((COUT, E, 3, 3), np.float64)
    we64 = we.astype(np.float64)
    for ch_ in range(3):
        for cw in range(3):
            Mcl[:, :, ch_, cw] = we64[:, :, sets[ch_], :][:, :, :, sets[cw]].sum((2, 3))
    T = (
        np.einsum("bce,cehw->bchw", vr, Mcl)
        + bm.astype(np.float64)[None, :, None, None]
        + be.astype(np.float64)[None, :, None, None]
    )

    # xrep[b, d*CIN+ci, (ch*C+j)*XROW + col] = xp[b, ci, 2*(ch*C+j)+d, col]
    xp = np.zeros((Bb, CIN, H + 2, XROW), np.float16)
    xp[:, :, 1 : H + 1, 1 : W + 1] = x.astype(np.float16)
    xrep = np.zeros((Bb, 4 * CIN, PLANE_PAD), np.float16)
    view = xrep[:, :, :PLANE].reshape(Bb, 4, CIN, PAIRS, XROW)
    for d in range(4):
        view[:, d] = xp[:, :, d : d + H : 2, :]

    # Fused weights: row q = kw*12 + d*CIN + ci, col u*128 + pair*64 + co
    pair_cls = {0: (0, 1), 1: (1, 1), 2: (1, 2)}
    wts = np.zeros((Bb, KP, 3, 128), np.float64)
    for u in range(3):
        for kw in range(KS):
            for pair in range(2):
                cols = slice(pair * 64, pair * 64 + 64)
                for d in range(4):
                    kh = d - pair
                    if 0 <= kh < KS:
                        for ci in range(CIN):
                            wts[:, kw * 12 + d * CIN + ci, u, cols] = wm[:, ci, kh, kw]
                cls = pair_cls[u][pair]
                wts[:, 36, u, cols] = T[:, :, cls, 0] - T[:, :, cls, 1]
                wts[:, 37, u, cols] = T[:, :, cls, 2] - T[:, :, cls, 1]
                wts[:, 38, u, cols] = T[:, :, cls, 1]
    wts = wts.reshape(Bb, KP, 3 * 128).astype(np.float16)

    stat = np.zeros((3, CBLK), np.float16)
    stat[0, 0::XROW] = 1.0          # rhs col 0 of each window
    stat[1, W - 1 :: XROW] = 1.0    # rhs col W-1 of each window
    stat[2, :] = 1.0                # ones row (bias + interior border term)
    return xrep, wts, stat


def kernel(**inputs) -> np.ndarray:
    x = np.ascontiguousarray(np.asarray(inputs["x"], np.float32))
    v = np.asarray(inputs["extra_inputs"], np.float32)
    wm = np.asarray(inputs["w_main"], np.float32)
    bm = np.asarray(inputs["b_main"], np.float32)
    we = np.asarray(inputs["w_extra"], np.float32)
    be = np.asarray(inputs["b_extra"], np.float32)

    xrep, wts, stat = _host_prep(x, v, wm, bm, we, be)

    if "nc" not in _cache:
        _cache["nc"] = _build()
    nc = _cache["nc"]

    in_maps = [{"xrep": xrep[b], "wts": wts[b], "stat": stat} for b in range(B)]
    res = run_bass_kernel_spmd(nc, in_maps, list(range(NCORES)))
    outs = []
    for b in range(B):
        ob = res.results[b]["out"]  # [128, NCHUNK*OBLK] fp16
        ob = ob.reshape(2, 64, NCHUNK, C, W).transpose(1, 2, 3, 0, 4)
        outs.append(ob.reshape(COUT, H, W).astype(np.float32))
    return np.stack(outs)
